# revision 1
# baseline (speedup 1.0000x reference)
"""Trainium2 Bass kernel: 5-point Jacobi stencil with Dirichlet boundary.

out[b,0,i,j] = 0.25*(v[i-1,j]+v[i+1,j]+v[i,j-1]+v[i,j+1]) + cof*f[i,j]  (interior)
out boundary = 0, where v = u with boundary forced to 0, cof = -(1/1023)^2/4.

Sharding: data-parallel over batch, 2 images per core on 8 cores.

Per-core layout: image [1024,1024] -> SBUF tile [128 partitions, 8*1024],
partition p holds rows 8p..8p+7 (contiguous DMA lines). All stencil taps are
then same-partition free-dim shifts (+-1 horizontal, +-1024 vertical), except
the up-tap of row 8p (from partition p-1) and down-tap of row 8p+7 (from
partition p+1), materialized once per image as halo tiles via partition-
shifted SBUF->SBUF DMA copies.

The u load brings in only interior rows/cols; boundary positions are zeroed
by memsets issued *before* the load (disjoint regions), so the v-boundary
zeroing never serializes against the DMA.

Engines: DVE does t1 = l+r and the fused (0.25*s + fcof) combine
(scalar_tensor_tensor); Pool (GPSIMD) does t2 = u+d; ACT does fcof = cof*f.
The s = t1+t2 op alternates DVE/Pool for balance. No TensorEngine (fp32r
matmul truncates the streaming operand; fp32 matmul is 4 cyc/row).
"""
import numpy as np
import concourse.bacc as bacc
import concourse.bass as bass
import concourse.mybir as mybir
from concourse.tile import TileContext
from concourse.bass_utils import run_bass_kernel_spmd

N_CORES = 8
B_FULL = 16
H = 1024
W = 1024
IMGS = B_FULL // N_CORES  # images per core
P = 128                   # partitions
RPP = H // P              # rows per partition = 8
FREE = RPP * W            # 8192
PAD = 1
COF = float(np.float32(-((1.0 / 1023.0) ** 2) / 4.0))
F32 = mybir.dt.float32

_cache = {}


def _build(repeat=1, INTERIOR_LOAD=False, BUFS=2, CHUNK_ORDER=1, FHALF=0, SPOOL=(0, 4), USPLIT=0, OBUFS=None, TBUFS=None, FBUFS=None, PECHUNKS=(), HALODRAM=0, PREFETCH=0):
    nc = bacc.Bacc("TRN2", target_bir_lowering=False)
    u_d = nc.dram_tensor("u", [IMGS, 1, H, W], F32, kind="ExternalInput")
    f_d = nc.dram_tensor("f", [IMGS, 1, H, W], F32, kind="ExternalInput")
    o_d = nc.dram_tensor("out", [IMGS, 1, H, W], F32, kind="ExternalOutput")
    id_d = nc.dram_tensor("ident", [P, P], F32, kind="ExternalInput") if PECHUNKS else None

    add = mybir.AluOpType.add
    mult = mybir.AluOpType.mult

    n_imgs = IMGS * repeat

    with TileContext(nc) as tc:
        with (
            tc.tile_pool(name="upool", bufs=2) as upool,
            tc.tile_pool(name="halopool", bufs=2) as halopool,
            tc.tile_pool(name="fpool", bufs=(FBUFS or BUFS)) as fpool,
            tc.tile_pool(name="t1pool", bufs=(TBUFS or BUFS)) as t1pool,
            tc.tile_pool(name="t2pool", bufs=(TBUFS or BUFS)) as t2pool,
            tc.tile_pool(name="opool", bufs=(OBUFS or BUFS)) as opool,
            tc.tile_pool(name="zpool", bufs=1) as zpool,
            tc.tile_pool(name="pspool", bufs=2, space="PSUM") as pspool,
        ):
            if PECHUNKS:
                id_t = zpool.tile([P, P], F32, name="id_t")
                nc.sync.dma_start(out=id_t, in_=id_d[:, :])
            # zeros line used to zero partition-127 regions (engine ops may
            # not start at partition 127; DMA can write anywhere)
            zt = zpool.tile([1, W], F32, name="zt")
            nc.vector.memset(zt, 0.0)
            def issue_loads(ib):
                b = ib % IMGS
                u4 = u_d[b, 0, :, :]            # [1024, 1024] DRAM

                ut = upool.tile([P, FREE + 2 * PAD], F32, name=f"ut{ib}", tag="ut")
                utv = ut[:, PAD : PAD + FREE].rearrange("p (r j) -> p r j", j=W)

                if INTERIOR_LOAD:
                    # boundary zeroing independent of the u load (disjoint
                    # regions; issued first so it hides under the DMA)
                    nc.vector.memset(ut[:, 0:PAD], 0.0)
                    nc.vector.memset(ut[:, PAD + FREE : PAD + FREE + PAD], 0.0)
                    nc.vector.memset(utv[:, :, 0:1], 0.0)            # col 0
                    nc.vector.memset(utv[:, :, W - 1 : W], 0.0)      # col 1023
                    nc.vector.memset(ut[0:1, PAD : PAD + W], 0.0)    # row 0
                    nc.sync.dma_start(                               # row 1023
                        out=ut[127:128, PAD + 7 * W : PAD + FREE], in_=zt
                    )
                    # u load: interior rows 1..1022, interior cols 1..1022
                    nc.sync.dma_start(
                        out=utv[0:1, 1:RPP, 1 : W - 1], in_=u4[1:RPP, 1 : W - 1]
                    )
                    nc.sync.dma_start(
                        out=utv[1:127, :, 1 : W - 1],
                        in_=u4[RPP : RPP * 127, 1 : W - 1].rearrange(
                            "(p r) j -> p r j", r=RPP
                        ),
                    )
                    nc.sync.dma_start(
                        out=utv[127:128, 0 : RPP - 1, 1 : W - 1],
                        in_=u4[RPP * 127 : H - 1, 1 : W - 1],
                    )
                else:
                    # full contiguous load, then boundary memsets
                    u_r = u4.rearrange("(p r) j -> p (r j)", r=RPP)
                    if USPLIT:
                        # split at r=5 so the first chunk's taps (rows r<=4)
                        # are ready before the whole image lands
                        nc.sync.dma_start(
                            out=ut[:, PAD : PAD + 5 * W], in_=u_r[:, 0 : 5 * W]
                        )
                        nc.sync.dma_start(
                            out=ut[:, PAD + 5 * W : PAD + FREE],
                            in_=u_r[:, 5 * W : FREE],
                        )
                    else:
                        nc.sync.dma_start(out=ut[:, PAD : PAD + FREE], in_=u_r)
                    nc.vector.memset(ut[:, 0:PAD], 0.0)
                    nc.vector.memset(ut[:, PAD + FREE : PAD + FREE + PAD], 0.0)
                    nc.vector.memset(ut[0:1, PAD : PAD + W], 0.0)
                    nc.sync.dma_start(
                        out=ut[127:128, PAD + 7 * W : PAD + FREE], in_=zt
                    )
                    nc.vector.memset(utv[:, :, 0:1], 0.0)
                    nc.vector.memset(utv[:, :, W - 1 : W], 0.0)

                # --- halo tiles: uh[p] = v[row 8p-1], dh[p] = v[row 8p+8]
                uh = halopool.tile([P, W], F32, name=f"uh{ib}", tag="uh")
                dh = halopool.tile([P, W], F32, name=f"dh{ib}", tag="dh")
                nc.vector.memset(uh[0:1, :], 0.0)
                nc.sync.dma_start(out=dh[127:128, :], in_=zt)
                if HALODRAM:
                    # straight from DRAM (strided rows): no dependency on the
                    # completed u load; col-boundary contamination lands only
                    # in output boundary columns which are zeroed anyway
                    u4r = u4.rearrange("(p r) j -> p r j", r=RPP)
                    nc.sync.dma_start(out=uh[1:128, :], in_=u4r[0:127, RPP - 1 : RPP, :])
                    nc.sync.dma_start(out=dh[0:127, :], in_=u4r[1:128, 0:1, :])
                else:
                    nc.sync.dma_start(
                        out=uh[1:128, :], in_=ut[0:127, PAD + 7 * W : PAD + FREE]
                    )
                    nc.sync.dma_start(out=dh[0:127, :], in_=ut[1:128, PAD : PAD + W])
                return ut, utv, uh, dh

            def issue_chunks(ib, ut, utv, uh, dh):
                b = ib % IMGS
                f_img = f_d[b, 0, :, :].rearrange("(p r) j -> p (r j)", r=RPP)
                o_img = o_d[b, 0, :, :].rearrange("(p r) j -> p (r j)", r=RPP)
                orders = {
                    0: [(0, 2), (2, 2), (4, 2), (6, 2)],
                    1: [(2, 2), (4, 2), (6, 2), (0, 2)],
                    2: [(2, 2), (4, 2), (0, 2), (6, 2)],
                    3: [(4, 2), (2, 2), (6, 2), (0, 2)],
                    4: [(6, 2), (4, 2), (2, 2), (0, 2)],
                    5: [(2, 2), (0, 2), (4, 2), (6, 2)],
                    6: [(1, 1), (2, 1), (3, 1), (4, 1), (5, 1), (6, 1), (7, 1), (0, 1)],
                    7: [(1, 2), (3, 2), (5, 2), (7, 1), (0, 1)],
                }
                chunks = orders[CHUNK_ORDER]

                fhalves = {}
                if FHALF:
                    for hi in range(2):
                        fh = fpool.tile([P, 4 * W], F32, name=f"fh{ib}_{hi}",
                                        tag="fc", padded_shape=[P, 4 * W])
                        nc.sync.dma_start(
                            out=fh, in_=f_img[:, hi * 4 * W : (hi + 1) * 4 * W]
                        )
                        fhalves[hi] = fh

                for ci, (r0, nr) in enumerate(chunks):
                    cw = nr * W
                    base = PAD + r0 * W
                    if FHALF:
                        fh = fhalves[r0 // 4]
                        off = (r0 % 4) * W
                        fc = fh[:, off : off + cw]
                    else:
                        fc = fpool.tile([P, cw], F32, name=f"fc{ib}_{ci}", tag="fc",
                                        padded_shape=[P, 2 * W])
                        nc.sync.dma_start(out=fc, in_=f_img[:, r0 * W : r0 * W + cw])
                    t1 = t1pool.tile([P, cw], F32, name=f"t1_{ib}_{ci}", tag="t1",
                                     padded_shape=[P, 2 * W])
                    t2 = None
                    if not (r0 in PECHUNKS and 0 < r0 and r0 + nr < RPP):
                        t2 = t2pool.tile([P, cw], F32, name=f"t2_{ib}_{ci}", tag="t2",
                                         padded_shape=[P, 2 * W])
                    oc = opool.tile([P, cw], F32, name=f"oc{ib}_{ci}", tag="oc",
                                    padded_shape=[P, 2 * W])

                    # t1 = left + right taps (free-dim +-1)
                    nc.vector.tensor_add(
                        out=t1,
                        in0=ut[:, base - 1 : base - 1 + cw],
                        in1=ut[:, base + 1 : base + 1 + cw],
                    )

                    # t2 = up + down taps (free-dim +-1024, halos at r=0 / r=7)
                    t2_eng = nc.gpsimd
                    pe_chunk = r0 in PECHUNKS and 0 < r0 and r0 + nr < RPP
                    if pe_chunk:
                        # PE lane: accumulate both vertical taps into PSUM via
                        # exact fp32 identity matmuls (512-col windows)
                        pt = pspool.tile([P, cw], F32, name=f"pt{ib}_{ci}", tag="pt")
                        for wdw in range(cw // 512):
                            nc.tensor.matmul(
                                pt[:, wdw * 512 : (wdw + 1) * 512],
                                id_t,
                                ut[:, base - W + wdw * 512 : base - W + wdw * 512 + 512],
                                start=True, stop=False,
                            )
                            nc.tensor.matmul(
                                pt[:, wdw * 512 : (wdw + 1) * 512],
                                id_t,
                                ut[:, base + W + wdw * 512 : base + W + wdw * 512 + 512],
                                start=False, stop=True,
                            )
                    elif r0 == 0:
                        nc.gpsimd.tensor_add(
                            out=t2[:, 0:W], in0=uh, in1=ut[:, PAD + W : PAD + 2 * W]
                        )
                        if nr == 2:
                            nc.gpsimd.tensor_add(
                                out=t2[:, W:cw],
                                in0=ut[:, PAD : PAD + W],
                                in1=ut[:, PAD + 2 * W : PAD + 3 * W],
                            )
                    elif r0 + nr == RPP:
                        # chunk touching r=7: down-tap of r=7 comes from dh
                        if nr == 2:
                            t2_eng.tensor_add(
                                out=t2[:, 0:W],
                                in0=ut[:, PAD + 5 * W : PAD + 6 * W],
                                in1=ut[:, PAD + 7 * W : PAD + FREE],
                            )
                        t2_eng.tensor_add(
                            out=t2[:, cw - W : cw],
                            in0=ut[:, PAD + 6 * W : PAD + 7 * W],
                            in1=dh,
                        )
                    else:
                        t2_eng.tensor_add(
                            out=t2,
                            in0=ut[:, base - W : base - W + cw],
                            in1=ut[:, base + W : base + W + cw],
                        )

                    # s = t1 + t2 (in-place into t1); SPOOL chunks on Pool.
                    # PE chunks read t2 from PSUM (DVE only; Pool has no PSUM port)
                    if pe_chunk:
                        nc.vector.tensor_add(out=t1, in0=t1, in1=pt)
                    elif r0 in SPOOL:
                        nc.gpsimd.tensor_add(out=t1, in0=t1, in1=t2)
                    else:
                        nc.vector.tensor_add(out=t1, in0=t1, in1=t2)

                    # fcof = cof * f (ACT, in-place)
                    nc.scalar.mul(fc, fc, COF)

                    # out = 0.25*s + fcof (fused on DVE)
                    nc.vector.scalar_tensor_tensor(
                        out=oc, in0=t1, scalar=0.25, in1=fc, op0=mult, op1=add
                    )

                    # zero output boundary inside this chunk
                    ocv = oc.rearrange("p (r j) -> p r j", j=W)
                    nc.vector.memset(ocv[:, :, 0:1], 0.0)
                    nc.vector.memset(ocv[:, :, W - 1 : W], 0.0)
                    if r0 == 0:
                        nc.vector.memset(oc[0:1, 0:W], 0.0)              # row 0
                    if r0 + nr == RPP:
                        nc.sync.dma_start(out=oc[127:128, cw - W : cw], in_=zt)

                    nc.sync.dma_start(out=o_img[:, r0 * W : r0 * W + cw], in_=oc)

            if PREFETCH:
                staged = []
                for ib in range(n_imgs):
                    staged.append(issue_loads(ib))
                    if len(staged) > 1:
                        issue_chunks(ib - 1, *staged.pop(0))
                issue_chunks(n_imgs - 1, *staged.pop(0))
            else:
                for ib in range(n_imgs):
                    issue_chunks(ib, *issue_loads(ib))
    nc.finalize()
    return nc


import os as _os
def _knobs():
    return dict(
        USPLIT=int(_os.environ.get("K_USPLIT", "0")),
        OBUFS=int(_os.environ.get("K_OBUFS", "3")) or None,
        TBUFS=int(_os.environ.get("K_TBUFS", "0")) or None,
        FBUFS=int(_os.environ.get("K_FBUFS", "0")) or None,
    )
def _get_nc(repeat=1):
    key = (repeat, tuple(sorted(_knobs().items())))
    if key not in _cache:
        _cache[key] = _build(repeat, **_knobs())
    return _cache[key]


def _run(u, f, trace=False):
    u = np.ascontiguousarray(np.asarray(u, dtype=np.float32))
    f = np.ascontiguousarray(np.asarray(f, dtype=np.float32))
    nc = _get_nc()
    in_maps = [
        {"u": u[i * IMGS : (i + 1) * IMGS], "f": f[i * IMGS : (i + 1) * IMGS]}
        for i in range(N_CORES)
    ]
    res = run_bass_kernel_spmd(nc, in_maps, core_ids=list(range(N_CORES)), trace=trace)
    out = np.concatenate([r["out"] for r in res.results], axis=0)
    return out, res


def kernel(u, f, weight=None):
    out, _ = _run(u, f)
    return out



# revision 2
# speedup vs baseline: 489.9019x; 489.9019x over previous
"""Trainium2 Bass kernel: 5-point Jacobi stencil with Dirichlet boundary.

out[b,0,i,j] = 0.25*(v[i-1,j]+v[i+1,j]+v[i,j-1]+v[i,j+1]) + cof*f[i,j]
(interior; boundary = 0), v = u with boundary forced to 0,
cof = -(1/1023)^2/4. Data-parallel over batch: 2 images per core, 8 cores.

Per-core layout: image [1024,1024] -> SBUF tile [128 partitions, 8 rows each
+ 2 pad elems], partition p holds rows 8p..8p+7, loaded as ONE SWDGE DMA
that casts fp32->bf16 in flight (halves SBUF-side DMA bytes; DRAM side runs
at the 16-engine line rate either way).

The whole stencil runs on the otherwise-idle TensorEngine as accumulating
512-col identity matmuls into PSUM: left/right taps are +-1-element shifts
of the streaming operand, up/down taps are +-1024 shifts, the two
cross-partition taps (up-tap of local row 0 / down-tap of local row 7) use
partition-shift stationaries S_up/S_dn against the neighbor partition's edge
row, and the f term accumulates via a (4*cof*I) stationary. bf16 streams run
1 col/cycle; PSUM accumulates in fp32, so the only error is the bf16 input
rounding (~2e-3 relative, well inside the 2e-2 gate).

DVE only drains PSUM (oc = 0.25 * psum, fp32), applies the v-column
boundary corrections, and zeroes the output boundary; stores are plain fp32
HWDGE DMAs on the sync ring (parallel to the SWDGE load queue). No engine
ever writes into the u tile (avoids an engine-write -> PE-read ordering
hazard observed with strided memsets): boundary rows are zeroed by DMA from
a zero line, and the 1-elem pads hold garbage that only ever feeds output
boundary columns, which are overwritten after the drain:
    oc[:, r, 1]    -= 0.25 * u[:, r, 0]
    oc[:, r, 1022] -= 0.25 * u[:, r, 1023]

Like v6 (banded u load, PE identity-matmul stencil) but no engine ever
writes into ut: boundary rows 0/1023 are zeroed by DMA from a zero line,
the 1-elem pads hold garbage (they only feed output boundary columns that
are memset after the drain), and the v-column-boundary condition is applied
as post-drain corrections on oc:
    oc[:, r, 1]    -= 0.25 * u[:, r, 0]
    oc[:, r, 1022] -= 0.25 * u[:, r, 1023]
"""
import numpy as np
import concourse.bacc as bacc
import concourse.bass as bass
import concourse.mybir as mybir
from concourse.ap import AP
from concourse.tile import TileContext
from concourse.bass_utils import run_bass_kernel_spmd

N_CORES = 8
B_FULL = 16
H = 1024
W = 1024
IMGS = B_FULL // N_CORES
P = 128
RPP = H // P
FREE = RPP * W + 2
COF = float(np.float32(-((1.0 / 1023.0) ** 2) / 4.0))
F32 = mybir.dt.float32
BF16 = mybir.dt.bfloat16

add = mybir.AluOpType.add
mult = mybir.AluOpType.mult

_cache = {}


def _build(repeat=1, CHUNKS=4, NSLOT=0, DRAIN="dve", OD=F32, PSBUFS=2,
           FBUFS=2, OBUFS=4, WIN=512, USPLIT=1, FSPLIT=1):
    nc = bacc.Bacc("TRN2", target_bir_lowering=False)
    u_d = nc.dram_tensor("u", [IMGS, 1, H, W], F32, kind="ExternalInput")
    f_d = nc.dram_tensor("f", [IMGS, 1, H, W], F32, kind="ExternalInput")
    id_d = nc.dram_tensor("ident", [P, 3 * P], F32, kind="ExternalInput")
    nout = NSLOT if NSLOT else IMGS
    o_d = nc.dram_tensor("out", [nout, 1, H, W], F32, kind="ExternalOutput")

    n_imgs = IMGS * repeat
    slot = [0]
    o_eng = nc.gpsimd if OD != F32 else nc.sync
    drain_eng = nc.vector if DRAIN == "dve" else nc.scalar

    with TileContext(nc) as tc:
        with (
            tc.tile_pool(name="upool", bufs=2) as upool,
            tc.tile_pool(name="fpool", bufs=FBUFS) as fpool,
            tc.tile_pool(name="opool", bufs=OBUFS) as opool,
            tc.tile_pool(name="zpool", bufs=1) as zpool,
            tc.tile_pool(name="pspool", bufs=PSBUFS, space="PSUM") as pspool,
        ):
            state = {}

            def issue_uload(ib):
                b = ib % IMGS
                u4 = u_d[b, 0, :, :]
                ut = upool.tile([P, FREE], BF16, name=f"ut{ib}", tag="ut")
                u_r = u4.rearrange("(p r) j -> p (r j)", r=RPP)
                nsp = max(1, USPLIT)
                step = RPP * W // nsp
                for si in range(nsp):
                    nc.gpsimd.dma_start(
                        out=ut[:, 1 + si * step : 1 + (si + 1) * step],
                        in_=u_r[:, si * step : (si + 1) * step],
                    )
                return ut

            def issue_setup():
                ztc = zpool.tile([1, W], BF16, name="ztc")
                nc.vector.memset(ztc, 0.0)
                zto = zpool.tile([1, W], OD, name="zto")
                nc.vector.memset(zto, 0.0)
                id_f = zpool.tile([P, 3 * P], F32, name="id_f")
                nc.sync.dma_start(out=id_f, in_=id_d[:, :])
                id_b = zpool.tile([P, P], BF16, name="id_b")
                nc.vector.tensor_scalar(out=id_b, in0=id_f[:, 0:P], scalar1=1.0,
                                        scalar2=None, op0=mult)
                cid_b = zpool.tile([P, P], BF16, name="cid_b")
                nc.vector.tensor_scalar(out=cid_b, in0=id_f[:, 0:P],
                                        scalar1=4.0 * COF, scalar2=None, op0=mult)
                sup_b = zpool.tile([P, P], BF16, name="sup_b")
                nc.vector.tensor_scalar(out=sup_b, in0=id_f[:, P : 2 * P],
                                        scalar1=1.0, scalar2=None, op0=mult)
                sdn_b = zpool.tile([P, P], BF16, name="sdn_b")
                nc.vector.tensor_scalar(out=sdn_b, in0=id_f[:, 2 * P : 3 * P],
                                        scalar1=1.0, scalar2=None, op0=mult)
                state.update(ztc=ztc, zto=zto, id_b=id_b, cid_b=cid_b,
                             sup_b=sup_b, sdn_b=sdn_b)

            def issue_vrows(ut):
                # v rows 0 / 1023 zeroed via DMA (no engine writes into ut)
                nc.sync.dma_start(out=ut[0:1, 1 : W + 1], in_=state["ztc"])
                nc.sync.dma_start(
                    out=ut[127:128, 1 + 7 * W : 1 + 8 * W], in_=state["ztc"]
                )

            def issue_chunks(ib, ut):
                b = ib % IMGS
                f_img = f_d[b, 0, :, :].rearrange("(p r) j -> p (r j)", r=RPP)
                if NSLOT:
                    ob = slot[0]
                    slot[0] = (slot[0] + 1) % NSLOT
                else:
                    ob = b
                o_img = o_d[ob, 0, :, :].rearrange("(p r) j -> p (r j)", r=RPP)
                id_b, cid_b = state["id_b"], state["cid_b"]
                sup_b, sdn_b = state["sup_b"], state["sdn_b"]

                ft = fpool.tile([P, RPP * W], BF16, name=f"ft{ib}", tag="ft")
                fstep = RPP * W // max(1, FSPLIT)
                for sfi in range(max(1, FSPLIT)):
                    nc.gpsimd.dma_start(
                        out=ft[:, sfi * fstep : (sfi + 1) * fstep],
                        in_=f_img[:, sfi * fstep : (sfi + 1) * fstep],
                    )

                nr = RPP // CHUNKS
                cw = nr * W
                for ci in range(CHUNKS):
                    r0 = ci * nr
                    base = 1 + r0 * W
                    fc = ft[:, r0 * W : r0 * W + cw]
                    pt = pspool.tile([P, cw], F32, name=f"pt{ib}_{ci}", tag="pt")
                    oc = opool.tile([P, cw], OD, name=f"oc{ib}_{ci}", tag="oc")

                    up_lo = W if r0 == 0 else 0
                    dn_hi = cw - W if r0 + nr == RPP else cw
                    for w in range(cw // WIN):
                        s0, s1 = w * WIN, (w + 1) * WIN

                        def mm(dst0, dst1, stat, src_tile, src0, start=False,
                               stop=False):
                            nc.tensor.matmul(
                                pt[:, dst0:dst1], stat,
                                src_tile[:, src0 : src0 + (dst1 - dst0)],
                                start=start, stop=stop,
                                skip_group_check=True,
                            )
                        mm(s0, s1, id_b, ut, base + s0 - 1, start=True)
                        mm(s0, s1, id_b, ut, base + s0 + 1)
                        a, bnd = max(s0, up_lo), s1
                        if a < bnd:
                            mm(a, bnd, id_b, ut, base + a - W)
                        if r0 == 0 and s0 < W:
                            a, bnd = s0, min(s1, W)
                            mm(a, bnd, sup_b, ut, 1 + 7 * W + a)
                        a, bnd = s0, min(s1, dn_hi)
                        if a < bnd:
                            mm(a, bnd, id_b, ut, base + a + W)
                        if r0 + nr == RPP and s1 > cw - W:
                            a, bnd = max(s0, cw - W), s1
                            mm(a, bnd, sdn_b, ut, 1 + (a - (cw - W)))
                        mm(s0, s1, cid_b, fc, s0, stop=True)

                    drain_eng.tensor_scalar(out=oc, in0=pt, scalar1=0.25,
                                            scalar2=None, op0=mult)

                    # v-column boundary corrections (l-tap of col 1 read u
                    # col 0; r-tap of col 1022 read u col 1023 — subtract)
                    ocv = oc.rearrange("p (r j) -> p r j", j=W)
                    utc = ut[:, base : base + cw].rearrange(
                        "p (r j) -> p r j", j=W
                    )
                    nc.vector.scalar_tensor_tensor(
                        out=ocv[:, :, 1:2], in0=utc[:, :, 0:1], scalar=-0.25,
                        in1=ocv[:, :, 1:2], op0=mult, op1=add,
                    )
                    nc.vector.scalar_tensor_tensor(
                        out=ocv[:, :, W - 2 : W - 1],
                        in0=utc[:, :, W - 1 : W], scalar=-0.25,
                        in1=ocv[:, :, W - 2 : W - 1], op0=mult, op1=add,
                    )
                    # zero output boundary
                    nc.vector.memset(ocv[:, :, 0:1], 0.0)
                    nc.vector.memset(ocv[:, :, W - 1 : W], 0.0)
                    if r0 == 0:
                        nc.vector.memset(oc[0:1, 0:W], 0.0)
                    if r0 + nr == RPP:
                        o_eng.dma_start(out=oc[127:128, cw - W : cw],
                                        in_=state["zto"])

                    o_eng.dma_start(out=o_img[:, r0 * W : r0 * W + cw], in_=oc)

            ut0 = issue_uload(0)
            issue_setup()
            issue_vrows(ut0)
            issue_chunks(0, ut0)
            for ib in range(1, n_imgs):
                ut = issue_uload(ib)
                issue_vrows(ut)
                issue_chunks(ib, ut)
    nc.finalize()
    return nc


def _make_ident():
    ident = np.zeros((P, 3 * P), dtype=np.float32)
    ident[:, 0:P] = np.eye(P, dtype=np.float32)
    for p in range(1, P):
        ident[p - 1, P + p] = 1.0
    for p in range(P - 1):
        ident[p + 1, 2 * P + p] = 1.0
    return ident


def _get_nc(repeat=1, **kw):
    key = (repeat, tuple(sorted(kw.items())))
    if key not in _cache:
        _cache[key] = _build(repeat, **kw)
    return _cache[key]


def _in_maps(u, f):
    ident = _make_ident()
    return [
        {"u": u[i * IMGS : (i + 1) * IMGS], "f": f[i * IMGS : (i + 1) * IMGS],
         "ident": ident}
        for i in range(N_CORES)
    ]


def _run(u, f, trace=False, **kw):
    u = np.ascontiguousarray(np.asarray(u, dtype=np.float32))
    f = np.ascontiguousarray(np.asarray(f, dtype=np.float32))
    nc = _get_nc(**kw)
    res = run_bass_kernel_spmd(nc, _in_maps(u, f), core_ids=list(range(N_CORES)),
                               trace=trace)
    out = np.concatenate([r["out"] for r in res.results], axis=0)
    return out, res


def kernel(u, f, weight=None):
    out, _ = _run(u, f)
    return out
